# revision 1
# baseline (speedup 1.0000x reference)
"""LorentzConv2d Trainium2 kernel v3.

Full-input contract: kernel(x=[8,56,56,64], kernels=[64,64]) -> [8,56,56,64].
Data-parallel over batch: one image per NeuronCore (8 cores).

Per-core algorithm on the zero-padded 58x58 grid, linearized l = 58*gh+gw,
tiled l = 128*t + p (p = partition):
  u[l,o]   = sum_c x[l,c] g_c k[o,c]    (PE, f32; col O accumulates sx)
  D[l,o]   = acosh(u)^2 = ln(u + sqrt(u^2-1+g))^2   (ACT chain, bf16)
  Q[l,o]   = -box3x3(D^2) + 2 sum_d box_d( D_si * D_sj * G_d )   (PE bands)
  S1[l,o]  = box3x3(sx*D)
  out_o    = (S1/63) * exp(-0.5 ln(-Q)) ; out_0 = exp(0.5 ln(1+sum out_o^2))
Shifted fields come from SBUF->SBUF partition-shift DMA copies (2 pieces,
full-bandwidth descriptors). A difference basis S={0,1,2,58,60,116,118}
expresses every window-pair offset d as sj-si, so only 13 shifted copies are
needed; each pass's band offsets are shifted by -si to compensate.  Broadcast
multiplies use a width-2 replicated operand so the DVE 2x 16-bit mode engages.
"""

import os
import numpy as np

import concourse.bass as bass
import concourse.bacc as bacc
import concourse.tile as tile
from concourse import mybir
from concourse.bass_utils import run_bass_kernel_spmd

# Restrict activation tables to the two actually used; keeps ln+exp+square in
# one set so the scheduler emits exactly two LoadActFuncSets.
import concourse.bacc as _bacc_mod
from concourse.hw_specs import get_activation_tables as _orig_gat


def _gat(arch):
    # Keep every table at its original index (act_func_set_id is positional
    # in act_info.json), but empty the ones we don't want chosen.
    tabs = _orig_gat(arch)
    keep = {"sqrt_and_others", "natural_log_exp_and_others"}
    if keep <= set(tabs):
        return {k: (v if k in keep else set()) for k, v in tabs.items()}
    return tabs


_bacc_mod.get_activation_tables = _gat

F32 = mybir.dt.float32
BF16 = mybir.dt.bfloat16
AF = mybir.ActivationFunctionType
OP = mybir.AluOpType

# geometry
H = W = 56
C = 64
O = 64
GW = 58                  # padded grid width (58x58)
NT = 27                  # 128-row tiles covering 58*58=3364 (+ tail)
NP = NT * 128            # 3456
NT1 = NT + 1             # +1 zero tail tile for shifted reads
NPADF = 2                # leading zero pad tiles in f fields (sides j>=-2)
NTF = NT + 3             # f field tiles: 2 lead + NT + 1 trail
SQ_GUARD = 1e-4          # replaces the max(u, 1+eps) clamp inside sqrt

# (dh, dw) per positive window-pair offset d = 58*dh + dw
DELTAS = {1: (0, 1), 2: (0, 2), 56: (1, -2), 57: (1, -1), 58: (1, 0),
          59: (1, 1), 60: (1, 2), 114: (2, -2), 115: (2, -1), 116: (2, 0),
          117: (2, 1), 118: (2, 2)}
# d -> (si, sj) with d = sj - si, both in the copy basis
PAIRS = {1: (1, 2), 2: (0, 2), 56: (2, 58), 57: (1, 58), 58: (0, 58),
         59: (1, 60), 60: (0, 60), 114: (2, 116), 115: (1, 116),
         116: (0, 116), 117: (1, 118), 118: (0, 118)}
XSHIFTS = [2, 58, 60, 116, 118]   # unsigned x copies (xc_s)
GXSHIFTS = [1, 2]                 # signed (col0-negated) copies (gxc_s)
DSHIFTS = [1, 2, 58, 60, 116, 118]
# delta processing order: by when source copies are likely ready
DORDER = [2, 1, 58, 57, 56, 60, 59, 116, 115, 114, 118, 117]


def _interval(d):
    return range(max(-1, -1 - d), min(1, 1 - d) + 1)


def _build_passes():
    """pass = (name, dkey_or_None, coeff, box_offsets(after -si), target).
    Ordered to match field completion order (PE consumes passes in order):
    the first two deltas' fields finish before diag/s1 are emitted."""
    box33 = [58 * a + b for a in (-1, 0, 1) for b in (-1, 0, 1)]

    def dpass(d):
        dh, dw = DELTAS[d]
        si, _ = PAIRS[d]
        box = [58 * a + b - si for a in _interval(dh) for b in _interval(dw)]
        return (f"d{d}", d, 2.0, box, "q")

    passes = [dpass(DORDER[0]), dpass(DORDER[1]),
              ("diag", None, -1.0, box33, "q"),
              ("s1", None, 1.0, box33, "s")]
    for d in DORDER[2:]:
        passes.append(dpass(d))
    return passes


def _build_bands(passes):
    """Banded-Toeplitz matrices. T[m, i] = coeff iff the source row m of tile
    c+j supplies out row i:  m = i + t - 128j for t in box."""
    mats = []
    sides = []
    for (_, _, coeff, box, _) in passes:
        plist = []
        for j in (-2, -1, 0, 1):
            T = np.zeros((128, 128), dtype=np.float32)
            for t in set(box):
                dd = t - 128 * j
                if -127 <= dd <= 127:
                    idx = np.arange(max(0, dd), 128 + min(0, dd))
                    T[idx, idx - dd] = coeff
            if np.any(T):
                plist.append((j, len(mats)))
                mats.append(T)
        sides.append(plist)
    return np.stack(mats), sides


PASSES = _build_passes()
BANDS, PASS_SIDES = _build_bands(PASSES)
NB = BANDS.shape[0]
CHUNKS = [(0, 8), (8, 8), (16, 8), (24, 3)]


def _shift_copy(nc, dst, src, s, eng=None):
    """dst[p, 0:NT, :] = src rows l+s (l = 128t+p), via two partition-shifted
    SBUF->SBUF DMAs. src is [128, NT1, inner] with a zero tail tile."""
    eng = eng or nc.sync
    assert 0 < s < 128
    eng.dma_start(out=dst[0:128 - s, :, :], in_=src[s:128, 0:NT, :])
    eng.dma_start(out=dst[128 - s:128, :, :], in_=src[0:s, 1:NT + 1, :])


def _rep2(t, n_inner):
    """[128, NT, 2] tile viewed as [128, NT, n_inner/2, 2] via paired
    stride-1 reads (keeps the DVE 16-bit 2x mode on broadcast multiplies)."""
    return t[:].unsqueeze(2).to_broadcast([128, NT, n_inner // 2, 2])


def _as4(ap, n_inner):
    """[128, NT, n_inner] AP viewed as [128, NT, n_inner/2, 2]."""
    return ap.rearrange("p t (a b) -> p t a b", b=2)


def build_nc(reps=1):
    nc = bacc.Bacc(None)
    # x arrives host-padded onto the 58x58 grid (+ zero tail tile)
    x_in = nc.declare_dram_parameter("x", [NT1 * 128, C], F32, isOutput=False)
    x16_in = nc.declare_dram_parameter("x16", [NT1 * 128, C], BF16,
                                       isOutput=False)
    gk_in = nc.declare_dram_parameter("gk_ext", [C, O + 1], F32,
                                      isOutput=False)
    # bands stored partition-major on host: [p][band][m]
    bands_in = nc.declare_dram_parameter("bands", [128, NB, 128], BF16,
                                         isOutput=False)
    id_in = nc.declare_dram_parameter("ident", [128, 128], F32, isOutput=False)
    # full padded output field; host extracts the 56x56 interior
    out_ext = nc.declare_dram_parameter("out", [NP, O], F32, isOutput=True)

    with tile.TileContext(nc) as tc:
        for rep in range(reps):
            with (
                tc.tile_pool(name=f"sg{rep}", bufs=1) as sg,
                tc.tile_pool(name=f"pp{rep}", bufs=1) as pp,
            ):
                _one_rep(nc, tc, sg, pp, x_in, x16_in, gk_in,
                         bands_in, id_in, out_ext, rep)
    nc.finalize()
    return nc


def _one_rep(nc, tc, sg, pp, x_in, x16_in, gk_in, bands_in, id_in,
             out_ext, rep):
    r = f"r{rep}_"

    def T(shape, dt, name):
        return sg.tile(shape, dt, tag=r + name, name=r + name)

    # ---- input DMAs (sync queue): interleave the bulky f32 x load with the
    # bf16 copies/shifts so G products and PE transposes both start early
    gk_sb = T([C, O + 1], F32, "gk")
    nc.sync.dma_start(out=gk_sb[:], in_=gk_in[:])
    id_sb = T([128, 128], F32, "id")
    nc.sync.dma_start(out=id_sb[:], in_=id_in[:])
    x_sb = T([128, NT1, C], F32, "x_sb")
    xview = x_in.rearrange("(t p) c -> p t c", p=128)
    xgrp = [(0, 7), (7, 7), (14, 7), (21, 7)]

    def _xchunk(gi):
        t0, tn = xgrp[gi]
        nc.sync.dma_start(out=x_sb[:, t0:t0 + tn, :],
                          in_=xview[:, t0:t0 + tn, :])

    x16 = T([128, NT1, C], BF16, "x16")
    nc.sync.dma_start(out=x16[:],
                      in_=x16_in.rearrange("(t p) c -> p t c", p=128))
    _xchunk(0)
    # gx16 on-chip: cheaper than a serial DMA ahead of the shift copies
    gx16 = T([128, NT1, C], BF16, "gx16")
    nc.vector.tensor_copy(gx16[:], x16[:])
    nc.vector.tensor_scalar_mul(gx16[:, :, 0], gx16[:, :, 0], -1.0)
    xc = {0: x16}
    gxc = {0: gx16}
    shift_jobs = ([("x", 2)] + [("gx", s) for s in GXSHIFTS]
                  + [("x", s) for s in XSHIFTS if s != 2])
    for i, (kind, s) in enumerate(shift_jobs):
        if i == 1:
            _xchunk(1)
        if i == 3:
            _xchunk(2)
        if i == 5:
            _xchunk(3)
        src, dstmap = (x16, xc) if kind == "x" else (gx16, gxc)
        dstmap[s] = T([128, NT, C], BF16, f"{kind}c{s}")
        _shift_copy(nc, dstmap[s], src, s)
    bands_sb = T([128, NB, 128], BF16, "bands")
    nc.sync.dma_start(out=bands_sb[:], in_=bands_in[:])

    # ---- phase A: transposes, u matmuls, dists
    ugroups = [(0, 7), (7, 7), (14, 7), (21, 6)]
    with (
        tc.tile_pool(name=r + "psA", bufs=1, space="PSUM") as psA,
        tc.tile_pool(name=r + "psT", bufs=3, space="PSUM") as psT,
    ):
        xT = T([64, NT, 128], F32, "xT")
        psu_g = [psA.tile([128, 7, O + 1], F32, tag=f"{r}psu{i}",
                          name=f"{r}psu{i}") for i in range(4)]
        for gi, (t0, tn) in enumerate(ugroups):
            for i in range(tn):
                tl = t0 + i
                xt_ps = psT.tile([C, 128], F32)
                nc.tensor.transpose(xt_ps[:], x_sb[:, tl, :], id_sb[:])
                nc.scalar.copy(xT[:, tl, :], xt_ps[:])
                nc.tensor.matmul(psu_g[gi][:, i, :], xT[:, tl, :], gk_sb[:],
                                 start=True, stop=True)

        # dists: sq = u^2 ; rt = sqrt(sq - 1 + g) ; D = ln(u + rt)^2
        d16 = T([128, NT1, O], BF16, "d16")
        nc.vector.memset(d16[:, NT, :], 0.0)
        sx_sb = T([128, NT], F32, "sx")
        cm1g = T([128, 1], F32, "cm1g")
        nc.gpsimd.memset(cm1g[:], -1.0 + SQ_GUARD)
        clnb = T([128, 1], F32, "clnb")
        nc.gpsimd.memset(clnb[:], 1e-30)
        sq = pp.tile([128, NT, O], F32, tag="big0", name=r + "sq")
        for gi, (t0, tn) in enumerate(ugroups):
            nc.scalar.activation(sq[:, t0:t0 + tn, :], psu_g[gi][:, :tn, 0:O],
                                 AF.Square)
            nc.scalar.copy(sx_sb[:, t0:t0 + tn], psu_g[gi][:, :tn, O])
        rl1 = pp.tile([128, NT, O], F32, tag="big1", name=r + "rl1")
        nc.scalar.activation(rl1[:], sq[:], AF.Relu, bias=cm1g[:])
        rt = pp.tile([128, NT, O], F32, tag="big2", name=r + "rt")
        nc.scalar.activation(rt[:], rl1[:], AF.Sqrt)
        # vv = u + rt - 1; acosh(u) = ln(1 + relu(vv)) — relu zeroes the
        # all-zero pad rows (u = rt = 0) so their D is exactly 0.
        vv = pp.tile([128, NT, O], F32, tag="big0", name=r + "vv")
        for gi, (t0, tn) in enumerate(ugroups):
            nc.vector.scalar_tensor_tensor(
                out=vv[:, t0:t0 + tn, :], in0=psu_g[gi][:, :tn, 0:O],
                scalar=-1.0, in1=rt[:, t0:t0 + tn, :], op0=OP.add,
                op1=OP.add)
        rl = pp.tile([128, NT, O], F32, tag="big1", name=r + "rl")
        nc.scalar.activation(rl[:], vv[:], AF.Relu)
        lnv = pp.tile([128, NT, O], F32, tag="big0", name=r + "lnv")
        nc.scalar.activation(lnv[:], rl[:], AF.Ln, bias=1.0)
        nc.scalar.activation(d16[:, 0:NT, :], lnv[:], AF.Square)
        sx2 = T([128, NT, 2], BF16, "sx2")
        nc.gpsimd.tensor_copy(sx2[:], sx_sb[:].unsqueeze(2).to_broadcast(
            [128, NT, 2]))

    # ---- shifted D copies
    dc = {0: d16}
    for s in DSHIFTS:
        dc[s] = T([128, NT, O], BF16, f"dc{s}")
        _shift_copy(nc, dc[s], d16, s)

    # ---- fields: per-delta pipeline on DVE with G-product lookahead so
    # each field completes as early as possible (PE consumes them in order)
    fields = {}

    def new_field(key):
        f = sg.tile([128, NTF, O], BF16, tag=f"{r}f{key}", name=f"{r}f{key}")
        nc.gpsimd.memset(f[:, 0:NPADF, :], 0.0)
        nc.gpsimd.memset(f[:, NPADF + NT:, :], 0.0)
        fields[key] = f
        return f

    g2 = {}
    tg = {}
    LOOK = 3

    def emit_prod(i):
        d = DORDER[i]
        si, sj = PAIRS[d]
        t = pp.tile([128, NT, C], BF16, tag=f"tg{i % (LOOK + 1)}",
                    name=f"{r}tg{d}")
        nc.vector.tensor_mul(t[:], gxc[si][:, 0:NT, :], xc[sj][:, 0:NT, :])
        tg[d] = t

    for i in range(LOOK):
        emit_prod(i)
    for i, d in enumerate(DORDER):
        if i + LOOK < len(DORDER):
            emit_prod(i + LOOK)
        t = tg[d]
        w = C // 2
        lvl = 0
        while w >= 2:  # bf16 halving levels on DVE (2x mode)
            o_t = pp.tile([128, NT, w], BF16, tag=f"tr{i % 2}_{lvl}",
                          name=f"{r}tr{d}_{lvl}")
            with nc.allow_low_precision(reason="bf16 tree partials"):
                nc.vector.tensor_add(o_t[:], t[:, :, 0:w], t[:, :, w:2 * w])
            t = o_t
            w //= 2
            lvl += 1
        g = T([128, NT, 2], BF16, f"g{d}")
        with nc.allow_low_precision(reason="bf16 G"):
            nc.vector.tensor_add(
                g[:], t[:, :, 0:1].to_broadcast([128, NT, 2]),
                t[:, :, 1:2].to_broadcast([128, NT, 2]))
        g2[d] = g
        si, sj = PAIRS[d]
        t2 = pp.tile([128, NT, O], BF16, tag=f"t2{i % 2}", name=f"{r}t2{d}")
        nc.vector.tensor_mul(t2[:], dc[si][:, 0:NT, :], dc[sj][:, 0:NT, :])
        f = new_field(f"d{d}")
        nc.vector.tensor_mul(_as4(f[:, NPADF:NPADF + NT, :], O),
                             _as4(t2[:], O), _rep2(g2[d], O))
        if i == 1:
            fdiag = new_field("diag")
            nc.vector.tensor_mul(fdiag[:, NPADF:NPADF + NT, :],
                                 d16[:, 0:NT, :], d16[:, 0:NT, :])
            fs1 = new_field("s1")
            nc.vector.tensor_mul(_as4(fs1[:, NPADF:NPADF + NT, :], O),
                                 _as4(d16[:, 0:NT, :], O), _rep2(sx2, O))

    # ---- pass-major banded box matmuls (all 4 chunks' PSUM live)
    osb = T([128, NT, O], F32, "osb")
    with (
        tc.tile_pool(name=r + "psQ", bufs=1, space="PSUM") as psQ,
        tc.tile_pool(name=r + "psS", bufs=1, space="PSUM") as psS,
    ):
        ps_q = [psQ.tile([128, cw, O], F32, tag=f"{r}psq{ci}",
                         name=f"{r}psq{ci}") for ci, (c0, cw) in
                enumerate(CHUNKS)]
        ps_s = [psS.tile([128, cw, O], F32, tag=f"{r}pss{ci}",
                         name=f"{r}pss{ci}") for ci, (c0, cw) in
                enumerate(CHUNKS)]
        wq = [0] * len(CHUNKS)
        ws = [0] * len(CHUNKS)
        nwq = sum(len(PASS_SIDES[pi]) for pi, p in enumerate(PASSES)
                  if p[4] == "q")
        nws = sum(len(PASS_SIDES[pi]) for pi, p in enumerate(PASSES)
                  if p[4] == "s")

        def box_half(chunk_ids):
            """Pass-major over a half of the chunks: PE consumes each field
            as soon as it is built; the other half's phase D overlaps."""
            for pi, (pname, dkey, coeff, box, tgt_kind) in enumerate(PASSES):
                fkey = "diag" if pname == "diag" else (
                    "s1" if pname == "s1" else f"d{dkey}")
                f = fields[fkey]
                for (j, bi) in PASS_SIDES[pi]:
                    for ci in chunk_ids:
                        c0, cw = CHUNKS[ci]
                        if tgt_kind == "q":
                            tgt, first, last = ps_q[ci], wq[ci] == 0, \
                                wq[ci] == nwq - 1
                            wq[ci] += 1
                        else:
                            tgt, first, last = ps_s[ci], ws[ci] == 0, \
                                ws[ci] == nws - 1
                            ws[ci] += 1
                        nc.tensor.matmul(
                            tgt[:], bands_sb[:, bi, :],
                            f[:, NPADF + c0 + j:NPADF + c0 + j + cw, :],
                            start=first, stop=last, skip_group_check=True)

        # ---- normalize/emit per chunk: rr = (-Q)^-1/2 via ln+exp
        def phase_d(ci):
            c0, cw = CHUNKS[ci]
            lnq = pp.tile([128, cw, O], F32, tag=f"lnq{ci % 2}", name=f"{r}lnq{ci}")
            nc.scalar.activation(lnq[:], ps_q[ci][:], AF.Ln, scale=-1.0,
                                 bias=clnb[:])
            rr = pp.tile([128, cw, O], F32, tag=f"rr{ci % 2}", name=f"{r}rr{ci}")
            nc.scalar.activation(rr[:], lnq[:], AF.Exp, scale=-0.5)
            nc.vector.scalar_tensor_tensor(
                out=osb[:, c0:c0 + cw, :], in0=ps_s[ci][:],
                scalar=1.0 / 63.0, in1=rr[:], op0=OP.mult, op1=OP.mult)
            s2 = pp.tile([128, cw, O - 1], F32, tag=f"s2{ci % 2}", name=f"{r}s2{ci}")
            nc.scalar.activation(s2[:], osb[:, c0:c0 + cw, 1:O], AF.Square)
            red = pp.tile([128, cw], F32, tag=f"red{ci % 2}", name=f"{r}red{ci}")
            nc.vector.tensor_reduce(red[:], s2[:], axis=mybir.AxisListType.X,
                                    op=OP.add)
            ln0 = pp.tile([128, cw], F32, tag=f"ln0{ci % 2}", name=f"{r}ln0{ci}")
            nc.scalar.activation(ln0[:], red[:], AF.Ln, bias=1.0)
            nc.scalar.activation(osb[:, c0:c0 + cw, 0], ln0[:], AF.Exp,
                                 scale=0.5)
            oview = out_ext[128 * c0:128 * (c0 + cw), :].rearrange(
                "(t p) c -> p t c", p=128)
            # cols 1..63 are final after the stt; ship them while the out0
            # subchain (s2/red/ln0/exp0) still runs, then the tiny col-0 DMA
            nc.sync.dma_start(out=oview[:, :, 1:O], in_=osb[:, c0:c0 + cw, 1:O])
            nc.sync.dma_start(out=oview[:, :, 0:1], in_=osb[:, c0:c0 + cw, 0:1])

        box_half([0, 1])
        phase_d(0)
        box_half([2])
        phase_d(1)
        box_half([3])
        phase_d(2)
        phase_d(3)


_NC_CACHE = None


def _get_nc():
    global _NC_CACHE
    if _NC_CACHE is None:
        _NC_CACHE = build_nc()
    return _NC_CACHE


def host_consts(kernels):
    # u = -l_inner(x,k) = x0*k0 - sum_{c>=1} x_c*k_c ; col O is sum_{c>=1} x_c
    gk_ext = np.zeros((C, O + 1), dtype=np.float32)
    gk_ext[:, :O] = kernels.astype(np.float32).T
    gk_ext[1:, :O] *= -1.0
    gk_ext[1:, O] = 1.0
    return gk_ext


def pad_image(img):
    """[56,56,64] -> host-padded [NT1*128, 64] on the 58x58 grid."""
    xp = np.zeros((NT1 * 128, C), dtype=np.float32)
    grid = xp[:GW * GW].reshape(GW, GW, C)
    grid[1:57, 1:57] = img
    return xp


def unpad_out(o):
    """[NP,64] padded field -> [56,56,64] interior."""
    return o[:GW * GW].reshape(GW, GW, O)[1:57, 1:57]


def core_inputs(x, kernels, core=0):
    import ml_dtypes
    xp = pad_image(np.asarray(x[core], dtype=np.float32))
    x16 = xp.astype(ml_dtypes.bfloat16)
    return {
        "x": xp,
        "x16": x16,
        "gk_ext": np.ascontiguousarray(host_consts(kernels)),
        "bands": np.ascontiguousarray(
            BANDS.transpose(1, 0, 2).astype(ml_dtypes.bfloat16)),
        "ident": np.eye(128, dtype=np.float32),
    }


def kernel(x, kernels):
    x = np.asarray(x, dtype=np.float32)
    kernels = np.asarray(kernels, dtype=np.float32)
    B = x.shape[0]
    assert x.shape == (B, H, W, C) and B == 8, x.shape
    nc = _get_nc()
    in_maps = [core_inputs(x, kernels, core=i) for i in range(8)]
    res = run_bass_kernel_spmd(nc, in_maps, core_ids=list(range(8)),
                               trace=bool(int(os.environ.get("KTRACE", "0"))))
    if res.exec_time_ns is not None:
        print(f"HW exec time: {res.exec_time_ns} ns")
    out = np.stack([unpad_out(res.results[i]["out"]) for i in range(8)])
    return out.astype(np.float32)

